# revision 23
# baseline (speedup 1.0000x reference)
"""ExplaiNN (dense_cnn) Trainium2 Bass kernel, 8-core SPMD.

Pipeline per reference:
  conv1d(4->300 units, K=19) + BN1 + exp + maxpool(7) -> per-unit fc1 (83->100)
  + BN2 + relu -> per-unit fc2 (100->1) + BN3 + relu -> final linear (300->2).

Distribution: conv+pool batch-sharded (16 b/core, all units); the pooled
features move to unit-sharding (38 u/core, full batch 128) via THREE chunked
AllToAlls (unit chunks 48/128/128, smallest first so the exchange starts as
early as possible) that overlap the conv of later chunks.  Final [128,2]
partials are summed on host.

Layout tricks:
  - pooled features are transposed [unit, pos] -> [pos, unit] on the PE
    (identity matmul, bf16 PSUM out), 4 batch elems per PSUM tile, one
    strided scalar-engine copy per 4 -> no DMA transposes;
  - conv columns split 504+77 so pool windows align with PSUM banks and the
    window-max is one big DVE reduce (tails of 4 batch elems share a bank);
  - fc1 keeps batch on the output partition dim (pooled cols stationary);
  - fc2 = elementwise mult + batched segment-reduce on DVE (tensor_tensor_
    reduce crashes the runtime, so mult/reduce pairs over 10-unit groups).

All BN affines are folded on host:
  praw = maxpool(conv_raw); pooled = exp(a1*praw + c1)        (a1>0)
  fc1 out[b,h] = pooled_col^T @ (a2*w1 | c2-row)  via pooled ones-row -> relu
  fc2 accum[b] = sum_h h2[b,h]*(a3*w2)[h] + c3    via ones-col -> relu
"""

import numpy as np
import ml_dtypes

B, N, L, K, C1 = 128, 300, 600, 19, 100
PS = 7
LC = 581          # conv outputs actually needed (l = 0..580; 83 pool windows)
LP = 83
NCLS = 2
EPS = 1e-5

NCORES = 8
BLOC = B // NCORES            # 16 batch per core in phase A
NPAD = 304                    # units padded to 8*38
ULOC = NPAD // NCORES         # 38 units per core in phase B
CK = 4 * K                    # 76 contraction rows for conv
# (unit chunk start, chunk size P, per-peer units usz); P = 8*usz.
# Smallest chunk first so its AllToAll posts earliest.
UCHUNKS = [(0, 128, 16), (128, 128, 16), (256, 48, 6)]
UOFF = [0, 16, 32]            # owned-unit index offset per chunk
FC2G = [(0, 10), (10, 20), (20, 30), (30, 38)]   # fc2 DVE groups

_CACHE = {}


def _build_bass():
    import concourse.bass as bass
    import concourse.bacc as bacc
    import concourse.mybir as mybir
    import concourse.tile as tile

    f32, bf16 = mybir.dt.float32, mybir.dt.bfloat16

    # Bacc (not plain Bass): its finalize() runs the wait-splitting passes
    # (move_matmul_waits_to_ldweights / generate_event_semaphores) that keep
    # every TPB command within its single hardware sync-wait slot.
    nc = bacc.Bacc("TRN2")
    xim = nc.declare_dram_parameter("xim", [CK, BLOC, LC], bf16, isOutput=False)
    wconv = nc.declare_dram_parameter("wconv", [CK, NPAD], bf16, isOutput=False)
    a1c1 = nc.declare_dram_parameter("a1c1", [128, 6], f32, isOutput=False)
    ident = nc.declare_dram_parameter("ident", [128, 128], bf16, isOutput=False)
    w1 = nc.declare_dram_parameter("w1", [LP + 1, ULOC * C1], bf16, isOutput=False)
    w2row = nc.declare_dram_parameter("w2row", [1, ULOC * (C1 + 1)], bf16,
                                      isOutput=False)
    fwrow = nc.declare_dram_parameter("fwrow", [1, NCLS * ULOC], f32,
                                      isOutput=False)
    out_part = nc.declare_dram_parameter("out_part", [B, NCLS], f32, isOutput=True)

    with tile.TileContext(nc) as tc:
        with (
            tc.tile_pool(name="dram", bufs=1, space="DRAM") as dram_pool,
            tc.tile_pool(name="singles", bufs=1) as singles,
            tc.tile_pool(name="scratch", bufs=1, space="PSUM") as scratch_pool,
            tc.tile_pool(name="tails", bufs=2, space="PSUM") as tails_pool,
            tc.tile_pool(name="psT", bufs=2, space="PSUM") as psT_pool,
        ):
            # DRAM exchange buffers: ONE AllToAll for all 304 units --
            # each collective has ~8-10us fixed cost, so chunked exchanges
            # lose; conv hides under the runtime's CC rendezvous anyway.
            # payload [peer, q(84), b(16), u(38)]; q row 83 = ones row
            p2p_in = dram_pool.tile([NCORES, LP + 1, BLOC, ULOC], bf16,
                                    name="p2p_in")
            p2p_out = dram_pool.tile([NCORES, LP + 1, BLOC, ULOC], bf16,
                                     name="p2p_out")

            # ---------------- parameter + input loads (SP queue) ----------
            wconv_sb = singles.tile([CK, NPAD], bf16)
            nc.sync.dma_start(out=wconv_sb, in_=wconv[:, :])
            im2 = [singles.tile([CK, 4, LC], bf16, name=f"im2_{t}")
                   for t in range(4)]

            def load_im2(t):
                nc.sync.dma_start(out=im2[t], in_=bass.AP(
                    tensor=xim, offset=t * 4 * LC,
                    ap=[[BLOC * LC, CK], [LC, 4], [1, LC]]))

            load_im2(0)
            a1c1_sb = singles.tile([128, 6], f32)
            nc.sync.dma_start(out=a1c1_sb, in_=a1c1[:, :])
            ident_sb = singles.tile([128, 128], bf16)
            nc.sync.dma_start(out=ident_sb, in_=ident[:, :])
            for t in range(1, 4):
                load_im2(t)
            w1_sb = singles.tile([LP + 1, ULOC * C1], bf16)
            nc.sync.dma_start(out=w1_sb, in_=w1[:, :])
            # broadcast the replicated rows over all 128 partitions in-DMA
            w2rep_sb = singles.tile([128, ULOC * (C1 + 1)], bf16)
            nc.sync.dma_start(
                out=w2rep_sb,
                in_=bass.AP(tensor=w2row, offset=0,
                            ap=[[0, 128], [1, ULOC * (C1 + 1)]]),
            )
            fwrep_sb = singles.tile([128, NCLS * ULOC], f32)
            nc.sync.dma_start(
                out=fwrep_sb,
                in_=bass.AP(tensor=fwrow, offset=0,
                            ap=[[0, 128], [1, NCLS * ULOC]]),
            )

            # ---------------- persistent SBUF state ----------------------
            praw, pexp, poolT = [], [], []
            for ci, (u0, P, usz) in enumerate(UCHUNKS):
                praw.append(singles.tile([128, BLOC, LP], f32, name=f"praw{ci}"))
                pexp.append(singles.tile([128, BLOC, LP + 1], bf16,
                                         name=f"pexp{ci}"))
                poolT.append(singles.tile([LP + 1, NCORES, BLOC, usz], bf16,
                                          name=f"poolT{ci}"))
                # whole-tile 1.0: col 83 becomes the transposed fc1 ones-row
                # (exp overwrites cols 0..82; contiguous memset is HW-safe)
                nc.gpsimd.memset(pexp[ci][:, :, :], 1.0)
            pTall = singles.tile([LP + 1, NCORES, BLOC, ULOC], bf16,
                                 name="pTall")

            h2T = singles.tile([128, ULOC * (C1 + 1)], bf16, name="h2T")
            # whole-tile 1.0: per-unit col 100 = ones (fc2 bias via c3 col)
            nc.gpsimd.memset(h2T[:, :], 1.0)
            junk = singles.tile([128, 10 * (C1 + 1)], f32, name="junk")
            h3 = singles.tile([128, ULOC], f32, name="h3")
            h3r = singles.tile([128, ULOC], f32, name="h3r")
            out_sb = singles.tile([B, NCLS], f32, name="out_sb")

            # PE matmuls only accept one sync wait; a dummy matmul reading a
            # freshly-DMA'd tile absorbs its semaphore so later real matmuls
            # need fewer.
            def absorb(tile_ap):
                s = scratch_pool.tile([2, 2], f32, name="dummy", tag="dummy")
                src = tile_ap.bitcast(bf16) if tile_ap.dtype != bf16 else tile_ap
                src = src[0:1, 0:2]
                nc.tensor.matmul(out=s, lhsT=src, rhs=src, start=True, stop=True)

            absorb(wconv_sb[0:1, 0:2])
            absorb(ident_sb[0:1, 0:2])

            # ---------------- phase emitters ------------------------------
            tails_tiles = {}

            def conv(psA, ci, blo, bhi):
                u0, P, usz = UCHUNKS[ci]
                lhsT = wconv_sb[:, u0:u0 + P]
                for b in range(blo, bhi):
                    g = b // 4
                    if b % 4 == 0:
                        tails_tiles[(ci, g)] = tails_pool.tile(
                            [128, 4 * 77], f32, name="tail", tag="tail")
                    tails = tails_tiles[(ci, g)]
                    ps = psA.tile([128, 504], f32, name="ps", tag="ps")
                    nc.tensor.matmul(
                        out=ps[0:P, :], lhsT=lhsT,
                        rhs=im2[b // 4][:, b % 4, 0:504],
                        start=True, stop=True,
                    )
                    nc.tensor.matmul(
                        out=tails[0:P, (b % 4) * 77:(b % 4 + 1) * 77], lhsT=lhsT,
                        rhs=im2[b // 4][:, b % 4, 504:LC],
                        start=True, stop=True,
                    )
                    nc.vector.reduce_max(
                        out=praw[ci][0:P, b, 0:72],
                        in_=ps[0:P, :].rearrange("p (q w) -> p q w", w=PS),
                        axis=mybir.AxisListType.X,
                        op=mybir.AluOpType.max,
                    )
                    if b % 4 == 3:
                        nc.vector.reduce_max(
                            out=praw[ci][0:P, 4 * (b // 4):4 * (b // 4) + 4, 72:83],
                            in_=tails[0:P, :].rearrange(
                                "p (c q w) -> p c q w", c=4, w=PS),
                            axis=mybir.AxisListType.X,
                            op=mybir.AluOpType.max,
                        )

            def exchange(ci):
                """exp -> PE transpose -> copy -> stage -> AllToAll -> unstage."""
                u0, P, usz = UCHUNKS[ci]
                # BN1+exp (pool commutes with monotone exp), in b-halves
                # so the transposes of the first half start sooner
                for h in range(2):
                    nc.scalar.activation(
                        out=pexp[ci][0:P, 8 * h:8 * h + 8, 0:LP],
                        in_=praw[ci][0:P, 8 * h:8 * h + 8, :],
                        func=mybir.ActivationFunctionType.Exp,
                        scale=a1c1_sb[0:P, ci:ci + 1],
                        bias=a1c1_sb[0:P, 3 + ci:3 + ci + 1],
                    )
                for g in range(4):
                    psT = psT_pool.tile([LP + 1, 512], bf16, name="psT", tag="psT")
                    for k in range(4):
                        b = 4 * g + k
                        nc.tensor.transpose(
                            psT[:, k * P:k * P + P],
                            pexp[ci][0:P, b, :],
                            ident_sb[0:P, 0:P],
                        )
                    # [84, (b,peer,u)] -> poolT [84, peer, b, u]
                    nc.scalar.copy(
                        out=poolT[ci][:, :, 4 * g:4 * g + 4, :],
                        in_=psT[:, 0:4 * P].rearrange(
                            "p (c e u) -> p e c u", c=4, u=usz),
                    )
                for j in range(NCORES):
                    nc.sync.dma_start(
                        out=bass.AP(
                            tensor=p2p_in.tensor,
                            offset=j * (LP + 1) * BLOC * ULOC + UOFF[ci],
                            ap=[[BLOC * ULOC, LP + 1], [ULOC, BLOC], [1, usz]],
                        ),
                        in_=poolT[ci][:, j, :, :],
                    )

            def unit_src(v):
                """pTall slice [84, 8, 16] holding pooled cols for owned unit v."""
                return pTall[:, :, :, v]

            # ---------------- emission schedule ---------------------------
            # PE order: conv0 | conv1a | T0 | conv1b conv2a | T1 | conv2b |
            # T2 | fc1 -- transposes land after the prior chunk's exp is
            # ready so the in-order PE queue never stalls long.
            with tc.tile_pool(name="psA", bufs=3, space="PSUM") as psA:
                conv(psA, 0, 0, BLOC)
                conv(psA, 1, 0, 8)
                exchange(0)
                conv(psA, 1, 8, BLOC)
                conv(psA, 2, 0, 8)
                exchange(1)
                conv(psA, 2, 8, BLOC)
            exchange(2)
            nc.gpsimd.collective_compute(
                "AllToAll",
                mybir.AluOpType.bypass,
                replica_groups=[list(range(NCORES))],
                ins=[p2p_in[:]],
                outs=[p2p_out[:]],
            )
            nc.gpsimd.dma_start(
                out=pTall[:, :, :, :],
                in_=bass.AP(
                    tensor=p2p_out.tensor, offset=0,
                    ap=[[BLOC * ULOC, LP + 1],
                        [(LP + 1) * BLOC * ULOC, NCORES],
                        [ULOC, BLOC], [1, ULOC]],
                ),
            )

            # ---------------- fc1 (PE) + relu (scalar) --------------------
            with tc.tile_pool(name="psB", bufs=2, space="PSUM") as psB:
                absorb(w1_sb[0:1, 0:2])
                groups = [(5 * g, min(5 * g + 5, ULOC)) for g in range(8)]
                for v0, v1 in groups:
                    ps = psB.tile([128, 500], f32, name="psf", tag="psf")
                    for v in range(v0, v1):
                        nc.tensor.matmul(
                            out=ps[:, (v - v0) * C1:(v - v0 + 1) * C1],
                            lhsT=unit_src(v),
                            rhs=w1_sb[:, v * C1:(v + 1) * C1],
                            start=True, stop=True,
                        )
                    nc.scalar.activation(
                        out=h2T.rearrange("p (u c) -> p u c", c=C1 + 1)[
                            :, v0:v1, 0:C1],
                        in_=ps[:, 0:(v1 - v0) * C1].rearrange(
                            "p (u c) -> p u c", c=C1),
                        func=mybir.ActivationFunctionType.Relu,
                    )

            # ---------------- fc2: mult + segment-reduce on DVE -----------
            for v0, v1 in FC2G:
                w = (v1 - v0) * (C1 + 1)
                nc.vector.tensor_mul(
                    out=junk[:, 0:w],
                    in0=h2T[:, v0 * (C1 + 1):v1 * (C1 + 1)],
                    in1=w2rep_sb[:, v0 * (C1 + 1):v1 * (C1 + 1)],
                )
                nc.vector.reduce_sum(
                    out=h3[:, v0:v1],
                    in_=junk[:, 0:w].rearrange("p (u c) -> p u c", c=C1 + 1),
                    axis=mybir.AxisListType.X,
                )

            # BN3 relu (affine folded into w2rep/c3), then final linear
            nc.vector.tensor_scalar_max(out=h3r, in0=h3, scalar1=0.0)
            for cls in range(NCLS):
                nc.vector.tensor_mul(
                    out=junk[:, 0:ULOC],
                    in0=h3r,
                    in1=fwrep_sb[:, cls * ULOC:(cls + 1) * ULOC],
                )
                nc.vector.reduce_sum(
                    out=out_sb[:, cls:cls + 1], in_=junk[:, 0:ULOC],
                    axis=mybir.AxisListType.X,
                )
            nc.sync.dma_start(out=out_part[:, :], in_=out_sb)

    nc.finalize()
    return nc


def _host_prep(inputs):
    """Fold BN affines, pad units to 304, build per-core input maps."""
    x = np.asarray(inputs["x"], np.float32)
    conv_w = np.asarray(inputs["conv_w"], np.float32)
    conv_b = np.asarray(inputs["conv_b"], np.float32)
    g1, b1 = np.asarray(inputs["bn1_g"], np.float32), np.asarray(inputs["bn1_b"], np.float32)
    m1, v1 = np.asarray(inputs["bn1_m"], np.float32), np.asarray(inputs["bn1_v"], np.float32)
    fc1_w, fc1_b = np.asarray(inputs["fc1_w"], np.float32), np.asarray(inputs["fc1_b"], np.float32)
    g2, b2 = np.asarray(inputs["bn2_g"], np.float32), np.asarray(inputs["bn2_b"], np.float32)
    m2, v2 = np.asarray(inputs["bn2_m"], np.float32), np.asarray(inputs["bn2_v"], np.float32)
    fc2_w, fc2_b = np.asarray(inputs["fc2_w"], np.float32), np.asarray(inputs["fc2_b"], np.float32)
    g3, b3 = np.asarray(inputs["bn3_g"], np.float32), np.asarray(inputs["bn3_b"], np.float32)
    m3, v3 = np.asarray(inputs["bn3_m"], np.float32), np.asarray(inputs["bn3_v"], np.float32)
    final_w = np.asarray(inputs["final_w"], np.float32)
    final_b = np.asarray(inputs["final_b"], np.float32)

    a1 = g1 / np.sqrt(v1 + EPS)                      # [300] > 0
    c1 = a1 * (conv_b - m1) + b1                     # [300]
    a2 = g2 / np.sqrt(v2 + EPS)                      # [300,100]
    c2 = b2 - a2 * m2 + a2 * fc1_b                   # [300,100]
    a3 = g3 / np.sqrt(v3 + EPS)                      # [300]
    c3 = a3 * (fc2_b - m3) + b3                      # [300]

    bf = ml_dtypes.bfloat16

    wconv = np.zeros((CK, NPAD), np.float32)
    wconv[:, :N] = conv_w.transpose(1, 2, 0).reshape(CK, N)

    a1p = np.ones(NPAD, np.float32)
    c1p = np.zeros(NPAD, np.float32)
    a1p[:N], c1p[:N] = a1, c1
    a1c1 = np.zeros((128, 6), np.float32)
    a1c1[:, 0:3] = 1.0
    for ci, (u0, P, usz) in enumerate(UCHUNKS):
        a1c1[0:P, ci] = a1p[u0:u0 + P]
        a1c1[0:P, 3 + ci] = c1p[u0:u0 + P]

    w1mod = np.zeros((NPAD, C1, LP), np.float32)
    w1mod[:N] = fc1_w * a2[:, :, None]
    c2p = np.zeros((NPAD, C1), np.float32)
    c2p[:N] = c2
    w2mod = np.zeros((NPAD, C1), np.float32)
    w2mod[:N] = fc2_w * a3[:, None]
    c3p = np.zeros(NPAD, np.float32)
    c3p[:N] = c3
    fwp = np.zeros((NCLS, NPAD), np.float32)
    fwp[:, :N] = final_w

    ident = np.eye(128, dtype=np.float32)

    in_maps = []
    for j in range(NCORES):
        # owned global units in UCHUNKS order (small chunk first)
        gs = []
        for ci, (u0, P, usz) in enumerate(UCHUNKS):
            gs += [u0 + j * usz + v for v in range(usz)]
        # fc1: per owned unit [84, 100]: rows 0..82 = a2*w1 (p-major),
        # row 83 = c2 (pairs with the pooled ones-row)
        w1c = np.zeros((LP + 1, ULOC * C1), np.float32)
        # fc2: per owned unit 101 cols: a3*w2 then c3 (pairs with h2T ones-col)
        w2c = np.zeros((ULOC * (C1 + 1),), np.float32)
        fwc = np.zeros((NCLS, ULOC), np.float32)
        for v, g in enumerate(gs):
            w1c[0:LP, v * C1:(v + 1) * C1] = w1mod[g].T
            w1c[LP, v * C1:(v + 1) * C1] = c2p[g]
            w2c[v * (C1 + 1):v * (C1 + 1) + C1] = w2mod[g]
            w2c[v * (C1 + 1) + C1] = c3p[g]
            fwc[:, v] = fwp[:, g]
        # im2col [c*19+k, b, l] = x[b, c, k+l]
        xc = x[j * BLOC:(j + 1) * BLOC]
        sw = np.lib.stride_tricks.sliding_window_view(xc, LC, axis=2)  # [16,4,20,581]
        xim = np.ascontiguousarray(
            sw[:, :, :K, :].transpose(1, 2, 0, 3).reshape(CK, BLOC, LC))
        in_maps.append({
            "xim": xim.astype(bf),
            "wconv": wconv.astype(bf),
            "a1c1": a1c1,
            "ident": ident.astype(bf),
            "w1": w1c.astype(bf),
            "w2row": w2c.astype(bf)[None, :],
            "fwrow": fwc.reshape(1, -1),
        })
    return in_maps, final_b


def kernel(**inputs):
    from concourse.bass_utils import run_bass_kernel_spmd

    if "nc" not in _CACHE:
        _CACHE["nc"] = _build_bass()
    nc = _CACHE["nc"]

    in_maps, final_b = _host_prep(inputs)
    res = run_bass_kernel_spmd(nc, in_maps, core_ids=list(range(NCORES)))
    out = np.zeros((B, NCLS), np.float32)
    for r in res.results:
        out += r["out_part"]
    out += final_b[None, :]
    return out


# revision 24
# speedup vs baseline: 1.7797x; 1.7797x over previous
"""ExplaiNN (dense_cnn) Trainium2 Bass kernel, 8-core SPMD.

Pipeline per reference:
  conv1d(4->300 units, K=19) + BN1 + exp + maxpool(7) -> per-unit fc1 (83->100)
  + BN2 + relu -> per-unit fc2 (100->1) + BN3 + relu -> final linear (300->2).

Distribution: conv+pool batch-sharded (16 b/core, all units); the pooled
features move to unit-sharding (38 u/core, full batch 128) via THREE chunked
AllToAlls (unit chunks 48/128/128, smallest first so the exchange starts as
early as possible) that overlap the conv of later chunks.  Final [128,2]
partials are summed on host.

Layout tricks:
  - pooled features are transposed [unit, pos] -> [pos, unit] on the PE
    (identity matmul, bf16 PSUM out), 4 batch elems per PSUM tile, one
    strided scalar-engine copy per 4 -> no DMA transposes;
  - conv columns split 504+77 so pool windows align with PSUM banks and the
    window-max is one big DVE reduce (tails of 4 batch elems share a bank);
  - fc1 keeps batch on the output partition dim (pooled cols stationary);
  - fc2 = elementwise mult + batched segment-reduce on DVE (tensor_tensor_
    reduce crashes the runtime, so mult/reduce pairs over 10-unit groups).

All BN affines are folded on host:
  praw = maxpool(conv_raw); pooled = exp(a1*praw + c1)        (a1>0)
  fc1 out[b,h] = pooled_col^T @ (a2*w1 | c2-row)  via pooled ones-row -> relu
  fc2 accum[b] = sum_h h2[b,h]*(a3*w2)[h] + c3    via ones-col -> relu
"""

import numpy as np
import ml_dtypes

B, N, L, K, C1 = 128, 300, 600, 19, 100
PS = 7
LC = 581          # conv outputs actually needed (l = 0..580; 83 pool windows)
LP = 83
NCLS = 2
EPS = 1e-5

NCORES = 8
BLOC = B // NCORES            # 16 batch per core in phase A
NPAD = 304                    # units padded to 8*38
ULOC = NPAD // NCORES         # 38 units per core in phase B
CK = 4 * K                    # 76 contraction rows for conv
# (unit chunk start, chunk size P, per-peer units usz); P = 8*usz.
# Smallest chunk first so its AllToAll posts earliest.
UCHUNKS = [(0, 128, 16), (128, 128, 16), (256, 48, 6)]
UOFF = [0, 16, 32]            # owned-unit index offset per chunk
FC2G = [(0, 10), (10, 20), (20, 30), (30, 38)]   # fc2 DVE groups
# element offset of each chunk's contiguous region inside a peer block
POFF = [0, (83 + 1) * 16 * 16, (83 + 1) * 16 * 32]
PBLK = (83 + 1) * 16 * 38     # peer block elements

_CACHE = {}


def _build_bass():
    import concourse.bass as bass
    import concourse.bacc as bacc
    import concourse.mybir as mybir
    import concourse.tile as tile

    f32, bf16 = mybir.dt.float32, mybir.dt.bfloat16

    # Bacc (not plain Bass): its finalize() runs the wait-splitting passes
    # (move_matmul_waits_to_ldweights / generate_event_semaphores) that keep
    # every TPB command within its single hardware sync-wait slot.
    nc = bacc.Bacc("TRN2")
    xim = nc.declare_dram_parameter("xim", [CK, BLOC, LC], bf16, isOutput=False)
    wconv = nc.declare_dram_parameter("wconv", [CK, NPAD], bf16, isOutput=False)
    a1c1 = nc.declare_dram_parameter("a1c1", [128, 6], f32, isOutput=False)
    ident = nc.declare_dram_parameter("ident", [128, 128], bf16, isOutput=False)
    w1 = nc.declare_dram_parameter("w1", [LP + 1, ULOC * C1], bf16, isOutput=False)
    w2row = nc.declare_dram_parameter("w2row", [1, ULOC * (C1 + 1)], bf16,
                                      isOutput=False)
    fwrow = nc.declare_dram_parameter("fwrow", [1, NCLS * ULOC], f32,
                                      isOutput=False)
    out_part = nc.declare_dram_parameter("out_part", [B, NCLS], f32, isOutput=True)

    with tile.TileContext(nc) as tc:
        with (
            tc.tile_pool(name="dram", bufs=1, space="DRAM") as dram_pool,
            tc.tile_pool(name="singles", bufs=1) as singles,
            tc.tile_pool(name="scratch", bufs=1, space="PSUM") as scratch_pool,
            tc.tile_pool(name="tails", bufs=2, space="PSUM") as tails_pool,
            tc.tile_pool(name="psT", bufs=2, space="PSUM") as psT_pool,
        ):
            # DRAM exchange buffers: ONE AllToAll for all 304 units --
            # each collective has ~8-10us fixed cost, so chunked exchanges
            # lose; conv hides under the runtime's CC rendezvous anyway.
            # payload [peer, q(84), b(16), u(38)]; q row 83 = ones row
            p2p_in = dram_pool.tile([NCORES, LP + 1, BLOC, ULOC], bf16,
                                    name="p2p_in")
            p2p_out = dram_pool.tile([NCORES, LP + 1, BLOC, ULOC], bf16,
                                     name="p2p_out")

            # ---------------- parameter + input loads (SP queue) ----------
            wconv_sb = singles.tile([CK, NPAD], bf16)
            nc.sync.dma_start(out=wconv_sb, in_=wconv[:, :])
            im2 = [singles.tile([CK, 4, LC], bf16, name=f"im2_{t}")
                   for t in range(4)]

            def load_im2(t):
                nc.sync.dma_start(out=im2[t], in_=bass.AP(
                    tensor=xim, offset=t * 4 * LC,
                    ap=[[BLOC * LC, CK], [LC, 4], [1, LC]]))

            load_im2(0)
            a1c1_sb = singles.tile([128, 6], f32)
            nc.sync.dma_start(out=a1c1_sb, in_=a1c1[:, :])
            ident_sb = singles.tile([128, 128], bf16)
            nc.sync.dma_start(out=ident_sb, in_=ident[:, :])
            for t in range(1, 4):
                load_im2(t)
            w1_sb = singles.tile([LP + 1, ULOC * C1], bf16)
            nc.sync.dma_start(out=w1_sb, in_=w1[:, :])
            # broadcast the replicated rows over all 128 partitions in-DMA
            w2rep_sb = singles.tile([128, ULOC * (C1 + 1)], bf16)
            nc.sync.dma_start(
                out=w2rep_sb,
                in_=bass.AP(tensor=w2row, offset=0,
                            ap=[[0, 128], [1, ULOC * (C1 + 1)]]),
            )
            fwrep_sb = singles.tile([128, NCLS * ULOC], f32)
            nc.sync.dma_start(
                out=fwrep_sb,
                in_=bass.AP(tensor=fwrow, offset=0,
                            ap=[[0, 128], [1, NCLS * ULOC]]),
            )

            # ---------------- persistent SBUF state ----------------------
            praw, pexp, poolT = [], [], []
            for ci, (u0, P, usz) in enumerate(UCHUNKS):
                praw.append(singles.tile([128, BLOC, LP], f32, name=f"praw{ci}"))
                pexp.append(singles.tile([128, BLOC, LP + 1], bf16,
                                         name=f"pexp{ci}"))
                poolT.append(singles.tile([LP + 1, NCORES, BLOC, usz], bf16,
                                          name=f"poolT{ci}"))
                # whole-tile 1.0: col 83 becomes the transposed fc1 ones-row
                # (exp overwrites cols 0..82; contiguous memset is HW-safe)
                nc.gpsimd.memset(pexp[ci][:, :, :], 1.0)
            pTall = []
            for ci, (u0, P, usz) in enumerate(UCHUNKS):
                pTall.append(singles.tile([LP + 1, NCORES, BLOC, usz], bf16,
                                          name=f"pTall{ci}"))

            h2T = singles.tile([128, ULOC * (C1 + 1)], bf16, name="h2T")
            # whole-tile 1.0: per-unit col 100 = ones (fc2 bias via c3 col)
            nc.gpsimd.memset(h2T[:, :], 1.0)
            junk = singles.tile([128, 10 * (C1 + 1)], f32, name="junk")
            h3 = singles.tile([128, ULOC], f32, name="h3")
            h3r = singles.tile([128, ULOC], f32, name="h3r")
            out_sb = singles.tile([B, NCLS], f32, name="out_sb")

            # PE matmuls only accept one sync wait; a dummy matmul reading a
            # freshly-DMA'd tile absorbs its semaphore so later real matmuls
            # need fewer.
            def absorb(tile_ap):
                s = scratch_pool.tile([2, 2], f32, name="dummy", tag="dummy")
                src = tile_ap.bitcast(bf16) if tile_ap.dtype != bf16 else tile_ap
                src = src[0:1, 0:2]
                nc.tensor.matmul(out=s, lhsT=src, rhs=src, start=True, stop=True)

            absorb(wconv_sb[0:1, 0:2])
            absorb(ident_sb[0:1, 0:2])

            # ---------------- phase emitters ------------------------------
            tails_tiles = {}

            def conv(psA, ci, blo, bhi):
                u0, P, usz = UCHUNKS[ci]
                lhsT = wconv_sb[:, u0:u0 + P]
                for b in range(blo, bhi):
                    g = b // 4
                    if b % 4 == 0:
                        tails_tiles[(ci, g)] = tails_pool.tile(
                            [128, 4 * 77], f32, name="tail", tag="tail")
                    tails = tails_tiles[(ci, g)]
                    ps = psA.tile([128, 504], f32, name="ps", tag="ps")
                    nc.tensor.matmul(
                        out=ps[0:P, :], lhsT=lhsT,
                        rhs=im2[b // 4][:, b % 4, 0:504],
                        start=True, stop=True,
                    )
                    nc.tensor.matmul(
                        out=tails[0:P, (b % 4) * 77:(b % 4 + 1) * 77], lhsT=lhsT,
                        rhs=im2[b // 4][:, b % 4, 504:LC],
                        start=True, stop=True,
                    )
                    nc.vector.reduce_max(
                        out=praw[ci][0:P, b, 0:72],
                        in_=ps[0:P, :].rearrange("p (q w) -> p q w", w=PS),
                        axis=mybir.AxisListType.X,
                        op=mybir.AluOpType.max,
                    )
                    if b % 4 == 3:
                        nc.vector.reduce_max(
                            out=praw[ci][0:P, 4 * (b // 4):4 * (b // 4) + 4, 72:83],
                            in_=tails[0:P, :].rearrange(
                                "p (c q w) -> p c q w", c=4, w=PS),
                            axis=mybir.AxisListType.X,
                            op=mybir.AluOpType.max,
                        )

            def exchange(ci):
                """exp -> PE transpose -> copy -> stage -> AllToAll -> unstage."""
                u0, P, usz = UCHUNKS[ci]
                # BN1+exp (pool commutes with monotone exp), in b-halves
                # so the transposes of the first half start sooner
                for h in range(2):
                    nc.scalar.activation(
                        out=pexp[ci][0:P, 8 * h:8 * h + 8, 0:LP],
                        in_=praw[ci][0:P, 8 * h:8 * h + 8, :],
                        func=mybir.ActivationFunctionType.Exp,
                        scale=a1c1_sb[0:P, ci:ci + 1],
                        bias=a1c1_sb[0:P, 3 + ci:3 + ci + 1],
                    )
                for g in range(4):
                    psT = psT_pool.tile([LP + 1, 512], bf16, name="psT", tag="psT")
                    for k in range(4):
                        b = 4 * g + k
                        nc.tensor.transpose(
                            psT[:, k * P:k * P + P],
                            pexp[ci][0:P, b, :],
                            ident_sb[0:P, 0:P],
                        )
                    # [84, (b,peer,u)] -> poolT [84, peer, b, u]
                    nc.scalar.copy(
                        out=poolT[ci][:, :, 4 * g:4 * g + 4, :],
                        in_=psT[:, 0:4 * P].rearrange(
                            "p (c e u) -> p e c u", c=4, u=usz),
                    )
                for j in range(NCORES):
                    nc.sync.dma_start(
                        out=bass.AP(
                            tensor=p2p_in.tensor,
                            offset=j * PBLK + POFF[ci],
                            ap=[[BLOC * usz, LP + 1], [usz, BLOC], [1, usz]],
                        ),
                        in_=poolT[ci][:, j, :, :],
                    )

            def unit_src(v):
                """pTall slice [84, 8, 16] holding pooled cols for owned unit v."""
                for ci in (2, 1, 0):
                    if v >= UOFF[ci]:
                        return pTall[ci][:, :, :, v - UOFF[ci]]

            # ---------------- emission schedule ---------------------------
            # PE order: conv0 | conv1a | T0 | conv1b conv2a | T1 | conv2b |
            # T2 | fc1 -- transposes land after the prior chunk's exp is
            # ready so the in-order PE queue never stalls long.
            with tc.tile_pool(name="psA", bufs=3, space="PSUM") as psA:
                conv(psA, 0, 0, BLOC)
                conv(psA, 1, 0, 8)
                exchange(0)
                conv(psA, 1, 8, BLOC)
                conv(psA, 2, 0, 8)
                exchange(1)
                conv(psA, 2, 8, BLOC)
            exchange(2)
            nc.gpsimd.collective_compute(
                "AllToAll",
                mybir.AluOpType.bypass,
                replica_groups=[list(range(NCORES))],
                ins=[p2p_in[:]],
                outs=[p2p_out[:]],
            )
            for ci, (u0, P, usz) in enumerate(UCHUNKS):
                nc.gpsimd.dma_start(
                    out=pTall[ci][:, :, :, :],
                    in_=bass.AP(
                        tensor=p2p_out.tensor, offset=POFF[ci],
                        ap=[[BLOC * usz, LP + 1], [PBLK, NCORES],
                            [usz, BLOC], [1, usz]],
                    ),
                )

            # ---------------- fc1 (PE) + relu (scalar) --------------------
            with tc.tile_pool(name="psB", bufs=2, space="PSUM") as psB:
                absorb(w1_sb[0:1, 0:2])
                groups = [(5 * g, min(5 * g + 5, ULOC)) for g in range(8)]
                for v0, v1 in groups:
                    ps = psB.tile([128, 500], f32, name="psf", tag="psf")
                    for v in range(v0, v1):
                        nc.tensor.matmul(
                            out=ps[:, (v - v0) * C1:(v - v0 + 1) * C1],
                            lhsT=unit_src(v),
                            rhs=w1_sb[:, v * C1:(v + 1) * C1],
                            start=True, stop=True,
                        )
                    nc.scalar.activation(
                        out=h2T.rearrange("p (u c) -> p u c", c=C1 + 1)[
                            :, v0:v1, 0:C1],
                        in_=ps[:, 0:(v1 - v0) * C1].rearrange(
                            "p (u c) -> p u c", c=C1),
                        func=mybir.ActivationFunctionType.Relu,
                    )

            # ---------------- fc2: mult + segment-reduce on DVE -----------
            for v0, v1 in FC2G:
                w = (v1 - v0) * (C1 + 1)
                nc.vector.tensor_mul(
                    out=junk[:, 0:w],
                    in0=h2T[:, v0 * (C1 + 1):v1 * (C1 + 1)],
                    in1=w2rep_sb[:, v0 * (C1 + 1):v1 * (C1 + 1)],
                )
                nc.vector.reduce_sum(
                    out=h3[:, v0:v1],
                    in_=junk[:, 0:w].rearrange("p (u c) -> p u c", c=C1 + 1),
                    axis=mybir.AxisListType.X,
                )

            # BN3 relu (affine folded into w2rep/c3), then final linear
            nc.vector.tensor_scalar_max(out=h3r, in0=h3, scalar1=0.0)
            for cls in range(NCLS):
                nc.vector.tensor_mul(
                    out=junk[:, 0:ULOC],
                    in0=h3r,
                    in1=fwrep_sb[:, cls * ULOC:(cls + 1) * ULOC],
                )
                nc.vector.reduce_sum(
                    out=out_sb[:, cls:cls + 1], in_=junk[:, 0:ULOC],
                    axis=mybir.AxisListType.X,
                )
            nc.sync.dma_start(out=out_part[:, :], in_=out_sb)

    nc.finalize()
    return nc


def _host_prep(inputs):
    """Fold BN affines, pad units to 304, build per-core input maps."""
    x = np.asarray(inputs["x"], np.float32)
    conv_w = np.asarray(inputs["conv_w"], np.float32)
    conv_b = np.asarray(inputs["conv_b"], np.float32)
    g1, b1 = np.asarray(inputs["bn1_g"], np.float32), np.asarray(inputs["bn1_b"], np.float32)
    m1, v1 = np.asarray(inputs["bn1_m"], np.float32), np.asarray(inputs["bn1_v"], np.float32)
    fc1_w, fc1_b = np.asarray(inputs["fc1_w"], np.float32), np.asarray(inputs["fc1_b"], np.float32)
    g2, b2 = np.asarray(inputs["bn2_g"], np.float32), np.asarray(inputs["bn2_b"], np.float32)
    m2, v2 = np.asarray(inputs["bn2_m"], np.float32), np.asarray(inputs["bn2_v"], np.float32)
    fc2_w, fc2_b = np.asarray(inputs["fc2_w"], np.float32), np.asarray(inputs["fc2_b"], np.float32)
    g3, b3 = np.asarray(inputs["bn3_g"], np.float32), np.asarray(inputs["bn3_b"], np.float32)
    m3, v3 = np.asarray(inputs["bn3_m"], np.float32), np.asarray(inputs["bn3_v"], np.float32)
    final_w = np.asarray(inputs["final_w"], np.float32)
    final_b = np.asarray(inputs["final_b"], np.float32)

    a1 = g1 / np.sqrt(v1 + EPS)                      # [300] > 0
    c1 = a1 * (conv_b - m1) + b1                     # [300]
    a2 = g2 / np.sqrt(v2 + EPS)                      # [300,100]
    c2 = b2 - a2 * m2 + a2 * fc1_b                   # [300,100]
    a3 = g3 / np.sqrt(v3 + EPS)                      # [300]
    c3 = a3 * (fc2_b - m3) + b3                      # [300]

    bf = ml_dtypes.bfloat16

    wconv = np.zeros((CK, NPAD), np.float32)
    wconv[:, :N] = conv_w.transpose(1, 2, 0).reshape(CK, N)

    a1p = np.ones(NPAD, np.float32)
    c1p = np.zeros(NPAD, np.float32)
    a1p[:N], c1p[:N] = a1, c1
    a1c1 = np.zeros((128, 6), np.float32)
    a1c1[:, 0:3] = 1.0
    for ci, (u0, P, usz) in enumerate(UCHUNKS):
        a1c1[0:P, ci] = a1p[u0:u0 + P]
        a1c1[0:P, 3 + ci] = c1p[u0:u0 + P]

    w1mod = np.zeros((NPAD, C1, LP), np.float32)
    w1mod[:N] = fc1_w * a2[:, :, None]
    c2p = np.zeros((NPAD, C1), np.float32)
    c2p[:N] = c2
    w2mod = np.zeros((NPAD, C1), np.float32)
    w2mod[:N] = fc2_w * a3[:, None]
    c3p = np.zeros(NPAD, np.float32)
    c3p[:N] = c3
    fwp = np.zeros((NCLS, NPAD), np.float32)
    fwp[:, :N] = final_w

    ident = np.eye(128, dtype=np.float32)

    in_maps = []
    for j in range(NCORES):
        # owned global units in UCHUNKS order (small chunk first)
        gs = []
        for ci, (u0, P, usz) in enumerate(UCHUNKS):
            gs += [u0 + j * usz + v for v in range(usz)]
        # fc1: per owned unit [84, 100]: rows 0..82 = a2*w1 (p-major),
        # row 83 = c2 (pairs with the pooled ones-row)
        w1c = np.zeros((LP + 1, ULOC * C1), np.float32)
        # fc2: per owned unit 101 cols: a3*w2 then c3 (pairs with h2T ones-col)
        w2c = np.zeros((ULOC * (C1 + 1),), np.float32)
        fwc = np.zeros((NCLS, ULOC), np.float32)
        for v, g in enumerate(gs):
            w1c[0:LP, v * C1:(v + 1) * C1] = w1mod[g].T
            w1c[LP, v * C1:(v + 1) * C1] = c2p[g]
            w2c[v * (C1 + 1):v * (C1 + 1) + C1] = w2mod[g]
            w2c[v * (C1 + 1) + C1] = c3p[g]
            fwc[:, v] = fwp[:, g]
        # im2col [c*19+k, b, l] = x[b, c, k+l]
        xc = x[j * BLOC:(j + 1) * BLOC]
        sw = np.lib.stride_tricks.sliding_window_view(xc, LC, axis=2)  # [16,4,20,581]
        xim = np.ascontiguousarray(
            sw[:, :, :K, :].transpose(1, 2, 0, 3).reshape(CK, BLOC, LC))
        in_maps.append({
            "xim": xim.astype(bf),
            "wconv": wconv.astype(bf),
            "a1c1": a1c1,
            "ident": ident.astype(bf),
            "w1": w1c.astype(bf),
            "w2row": w2c.astype(bf)[None, :],
            "fwrow": fwc.reshape(1, -1),
        })
    return in_maps, final_b


def kernel(**inputs):
    from concourse.bass_utils import run_bass_kernel_spmd

    if "nc" not in _CACHE:
        _CACHE["nc"] = _build_bass()
    nc = _CACHE["nc"]

    in_maps, final_b = _host_prep(inputs)
    res = run_bass_kernel_spmd(nc, in_maps, core_ids=list(range(NCORES)))
    out = np.zeros((B, NCLS), np.float32)
    for r in res.results:
        out += r["out_part"]
    out += final_b[None, :]
    return out


# revision 26
# speedup vs baseline: 2.4223x; 1.3611x over previous
"""ExplaiNN (dense_cnn) Trainium2 Bass kernel, 8-core SPMD.

Pipeline per reference:
  conv1d(4->300 units, K=19) + BN1 + exp + maxpool(7) -> per-unit fc1 (83->100)
  + BN2 + relu -> per-unit fc2 (100->1) + BN3 + relu -> final linear (300->2).

Distribution: conv+pool batch-sharded (16 b/core, all units); the pooled
features move to unit-sharding (38 u/core, full batch 128) via THREE chunked
AllToAlls (unit chunks 48/128/128, smallest first so the exchange starts as
early as possible) that overlap the conv of later chunks.  Final [128,2]
partials are summed on host.

Layout tricks:
  - pooled features are transposed [unit, pos] -> [pos, unit] on the PE
    (identity matmul, bf16 PSUM out), 4 batch elems per PSUM tile, one
    strided scalar-engine copy per 4 -> no DMA transposes;
  - conv columns split 504+77 so pool windows align with PSUM banks and the
    window-max is one big DVE reduce (tails of 4 batch elems share a bank);
  - fc1 keeps batch on the output partition dim (pooled cols stationary);
  - fc2 = elementwise mult + batched segment-reduce on DVE (tensor_tensor_
    reduce crashes the runtime, so mult/reduce pairs over 10-unit groups).

All BN affines are folded on host:
  praw = maxpool(conv_raw); pooled = exp(a1*praw + c1)        (a1>0)
  fc1 out[b,h] = pooled_col^T @ (a2*w1 | c2-row)  via pooled ones-row -> relu
  fc2 accum[b] = sum_h h2[b,h]*(a3*w2)[h] + c3    via ones-col -> relu
"""

import numpy as np
import ml_dtypes

B, N, L, K, C1 = 128, 300, 600, 19, 100
PS = 7
LC = 581          # conv outputs actually needed (l = 0..580; 83 pool windows)
LP = 83
NCLS = 2
EPS = 1e-5

NCORES = 8
BLOC = B // NCORES            # 16 batch per core in phase A
NPAD = 304                    # units padded to 8*38
ULOC = NPAD // NCORES         # 38 units per core in phase B
CK = 4 * K                    # 76 contraction rows for conv
# (unit chunk start, chunk size P, per-peer units usz); P = 8*usz.
# Smallest chunk first so its AllToAll posts earliest.
UCHUNKS = [(0, 128, 16), (128, 128, 16), (256, 48, 6)]
UOFF = [0, 16, 32]            # owned-unit index offset per chunk
FC2G = [(0, 10), (10, 20), (20, 30), (30, 38)]   # fc2 DVE groups
# element offset of each chunk's contiguous region inside a peer block
POFF = [0, (83 + 1) * 16 * 16, (83 + 1) * 16 * 32]
PBLK = (83 + 1) * 16 * 38     # peer block elements

_CACHE = {}


def _build_bass():
    import concourse.bass as bass
    import concourse.bacc as bacc
    import concourse.mybir as mybir
    import concourse.tile as tile

    f32, bf16 = mybir.dt.float32, mybir.dt.bfloat16

    # Bacc (not plain Bass): its finalize() runs the wait-splitting passes
    # (move_matmul_waits_to_ldweights / generate_event_semaphores) that keep
    # every TPB command within its single hardware sync-wait slot.
    nc = bacc.Bacc("TRN2")
    xim = nc.declare_dram_parameter("xim", [CK, BLOC, LC], bf16, isOutput=False)
    wconv = nc.declare_dram_parameter("wconv", [CK, NPAD], bf16, isOutput=False)
    a1c1 = nc.declare_dram_parameter("a1c1", [128, 6], f32, isOutput=False)
    ident = nc.declare_dram_parameter("ident", [128, 128], bf16, isOutput=False)
    w1 = nc.declare_dram_parameter("w1", [LP + 1, ULOC * C1], bf16, isOutput=False)
    w2row = nc.declare_dram_parameter("w2row", [1, ULOC * (C1 + 1)], bf16,
                                      isOutput=False)
    fwrow = nc.declare_dram_parameter("fwrow", [1, NCLS * ULOC], f32,
                                      isOutput=False)
    out_part = nc.declare_dram_parameter("out_part", [B, NCLS], f32, isOutput=True)

    with tile.TileContext(nc) as tc:
        with (
            tc.tile_pool(name="dram", bufs=1, space="DRAM") as dram_pool,
            tc.tile_pool(name="singles", bufs=1) as singles,
            tc.tile_pool(name="scratch", bufs=1, space="PSUM") as scratch_pool,
            tc.tile_pool(name="tails", bufs=2, space="PSUM") as tails_pool,
            tc.tile_pool(name="psT", bufs=2, space="PSUM") as psT_pool,
        ):
            # DRAM exchange buffers, one contiguous pair per unit chunk
            # (collectives require contiguous patterns).
            # payload [peer, q(84), b(16), usz]; q row 83 = ones row
            p2p_in, p2p_out = [], []
            for ci, (u0, P, usz) in enumerate(UCHUNKS):
                p2p_in.append(dram_pool.tile([NCORES, LP + 1, BLOC, usz], bf16,
                                             name=f"p2p_in{ci}"))
                p2p_out.append(dram_pool.tile([NCORES, LP + 1, BLOC, usz], bf16,
                                              name=f"p2p_out{ci}"))

            # ---------------- parameter + input loads (SP queue) ----------
            wconv_sb = singles.tile([CK, NPAD], bf16)
            nc.sync.dma_start(out=wconv_sb, in_=wconv[:, :])
            im2 = [singles.tile([CK, 4, LC], bf16, name=f"im2_{t}")
                   for t in range(4)]

            def load_im2(t):
                nc.sync.dma_start(out=im2[t], in_=bass.AP(
                    tensor=xim, offset=t * 4 * LC,
                    ap=[[BLOC * LC, CK], [LC, 4], [1, LC]]))

            load_im2(0)
            a1c1_sb = singles.tile([128, 6], f32)
            nc.sync.dma_start(out=a1c1_sb, in_=a1c1[:, :])
            ident_sb = singles.tile([128, 128], bf16)
            nc.sync.dma_start(out=ident_sb, in_=ident[:, :])
            for t in range(1, 4):
                load_im2(t)
            w1_sb = singles.tile([LP + 1, ULOC * C1], bf16)
            nc.sync.dma_start(out=w1_sb, in_=w1[:, :])
            # broadcast the replicated rows over all 128 partitions in-DMA
            w2rep_sb = singles.tile([128, ULOC * (C1 + 1)], bf16)
            nc.sync.dma_start(
                out=w2rep_sb,
                in_=bass.AP(tensor=w2row, offset=0,
                            ap=[[0, 128], [1, ULOC * (C1 + 1)]]),
            )
            fwrep_sb = singles.tile([128, NCLS * ULOC], f32)
            nc.sync.dma_start(
                out=fwrep_sb,
                in_=bass.AP(tensor=fwrow, offset=0,
                            ap=[[0, 128], [1, NCLS * ULOC]]),
            )

            # ---------------- persistent SBUF state ----------------------
            praw, pexp, poolT = [], [], []
            for ci, (u0, P, usz) in enumerate(UCHUNKS):
                praw.append(singles.tile([128, BLOC, LP], f32, name=f"praw{ci}"))
                pexp.append(singles.tile([128, BLOC, LP + 1], bf16,
                                         name=f"pexp{ci}"))
                poolT.append(singles.tile([LP + 1, NCORES, BLOC, usz], bf16,
                                          name=f"poolT{ci}"))
                # whole-tile 1.0: col 83 becomes the transposed fc1 ones-row
                # (exp overwrites cols 0..82; contiguous memset is HW-safe)
                nc.gpsimd.memset(pexp[ci][:, :, :], 1.0)
            pTall = []
            for ci, (u0, P, usz) in enumerate(UCHUNKS):
                pTall.append(singles.tile([LP + 1, NCORES, BLOC, usz], bf16,
                                          name=f"pTall{ci}"))

            h2T = singles.tile([128, ULOC * (C1 + 1)], bf16, name="h2T")
            # whole-tile 1.0: per-unit col 100 = ones (fc2 bias via c3 col)
            nc.gpsimd.memset(h2T[:, :], 1.0)
            junk = singles.tile([128, 10 * (C1 + 1)], f32, name="junk")
            h3 = singles.tile([128, ULOC], f32, name="h3")
            h3r = singles.tile([128, ULOC], f32, name="h3r")
            out_sb = singles.tile([B, NCLS], f32, name="out_sb")

            # PE matmuls only accept one sync wait; a dummy matmul reading a
            # freshly-DMA'd tile absorbs its semaphore so later real matmuls
            # need fewer.
            def absorb(tile_ap):
                s = scratch_pool.tile([2, 2], f32, name="dummy", tag="dummy")
                src = tile_ap.bitcast(bf16) if tile_ap.dtype != bf16 else tile_ap
                src = src[0:1, 0:2]
                nc.tensor.matmul(out=s, lhsT=src, rhs=src, start=True, stop=True)

            absorb(wconv_sb[0:1, 0:2])
            absorb(ident_sb[0:1, 0:2])

            # ---------------- phase emitters ------------------------------
            tails_tiles = {}

            def conv(psA, ci, blo, bhi):
                u0, P, usz = UCHUNKS[ci]
                lhsT = wconv_sb[:, u0:u0 + P]
                for b in range(blo, bhi):
                    g = b // 4
                    if b % 4 == 0:
                        tails_tiles[(ci, g)] = tails_pool.tile(
                            [128, 4 * 77], f32, name="tail", tag="tail")
                    tails = tails_tiles[(ci, g)]
                    ps = psA.tile([128, 504], f32, name="ps", tag="ps")
                    nc.tensor.matmul(
                        out=ps[0:P, :], lhsT=lhsT,
                        rhs=im2[b // 4][:, b % 4, 0:504],
                        start=True, stop=True,
                    )
                    nc.tensor.matmul(
                        out=tails[0:P, (b % 4) * 77:(b % 4 + 1) * 77], lhsT=lhsT,
                        rhs=im2[b // 4][:, b % 4, 504:LC],
                        start=True, stop=True,
                    )
                    nc.vector.reduce_max(
                        out=praw[ci][0:P, b, 0:72],
                        in_=ps[0:P, :].rearrange("p (q w) -> p q w", w=PS),
                        axis=mybir.AxisListType.X,
                        op=mybir.AluOpType.max,
                    )
                    if b % 4 == 3:
                        nc.vector.reduce_max(
                            out=praw[ci][0:P, 4 * (b // 4):4 * (b // 4) + 4, 72:83],
                            in_=tails[0:P, :].rearrange(
                                "p (c q w) -> p c q w", c=4, w=PS),
                            axis=mybir.AxisListType.X,
                            op=mybir.AluOpType.max,
                        )

            def exchange(ci):
                """exp -> PE transpose -> copy -> stage -> AllToAll -> unstage."""
                u0, P, usz = UCHUNKS[ci]
                # BN1+exp (pool commutes with monotone exp), in b-halves
                # so the transposes of the first half start sooner
                for h in range(2):
                    nc.scalar.activation(
                        out=pexp[ci][0:P, 8 * h:8 * h + 8, 0:LP],
                        in_=praw[ci][0:P, 8 * h:8 * h + 8, :],
                        func=mybir.ActivationFunctionType.Exp,
                        scale=a1c1_sb[0:P, ci:ci + 1],
                        bias=a1c1_sb[0:P, 3 + ci:3 + ci + 1],
                    )
                for g in range(4):
                    psT = psT_pool.tile([LP + 1, 512], bf16, name="psT", tag="psT")
                    for k in range(4):
                        b = 4 * g + k
                        nc.tensor.transpose(
                            psT[:, k * P:k * P + P],
                            pexp[ci][0:P, b, :],
                            ident_sb[0:P, 0:P],
                        )
                    # [84, (b,peer,u)] -> poolT [84, peer, b, u]
                    nc.scalar.copy(
                        out=poolT[ci][:, :, 4 * g:4 * g + 4, :],
                        in_=psT[:, 0:4 * P].rearrange(
                            "p (c e u) -> p e c u", c=4, u=usz),
                    )
                for j in range(NCORES):
                    nc.sync.dma_start(
                        out=p2p_in[ci][j, :, :, :],
                        in_=poolT[ci][:, j, :, :],
                    )
                nc.gpsimd.collective_compute(
                    "AllToAll",
                    mybir.AluOpType.bypass,
                    replica_groups=[list(range(NCORES))],
                    ins=[p2p_in[ci][:]],
                    outs=[p2p_out[ci][:]],
                )
                nc.gpsimd.dma_start(
                    out=pTall[ci][:, :, :, :],
                    in_=bass.AP(
                        tensor=p2p_out[ci].tensor, offset=0,
                        ap=[[BLOC * usz, LP + 1],
                            [(LP + 1) * BLOC * usz, NCORES],
                            [usz, BLOC], [1, usz]],
                    ),
                )

            def unit_src(v):
                """pTall slice [84, 8, 16] holding pooled cols for owned unit v."""
                for ci in (2, 1, 0):
                    if v >= UOFF[ci]:
                        return pTall[ci][:, :, :, v - UOFF[ci]]

            # ---------------- emission schedule ---------------------------
            # PE order: conv0 | conv1a | T0 | conv1b conv2a | T1 | conv2b |
            # T2 | fc1 -- transposes land after the prior chunk's exp is
            # ready so the in-order PE queue never stalls long.
            with tc.tile_pool(name="psA", bufs=3, space="PSUM") as psA:
                conv(psA, 0, 0, BLOC)
                conv(psA, 1, 0, 8)
                exchange(0)
                conv(psA, 1, 8, BLOC)
                conv(psA, 2, 0, 8)
                exchange(1)
                conv(psA, 2, 8, BLOC)
            exchange(2)

            # ---------------- fc1 (PE) + relu (scalar) --------------------
            with tc.tile_pool(name="psB", bufs=2, space="PSUM") as psB:
                absorb(w1_sb[0:1, 0:2])
                groups = [(5 * g, min(5 * g + 5, ULOC)) for g in range(8)]
                for v0, v1 in groups:
                    ps = psB.tile([128, 500], f32, name="psf", tag="psf")
                    for v in range(v0, v1):
                        nc.tensor.matmul(
                            out=ps[:, (v - v0) * C1:(v - v0 + 1) * C1],
                            lhsT=unit_src(v),
                            rhs=w1_sb[:, v * C1:(v + 1) * C1],
                            start=True, stop=True,
                        )
                    nc.scalar.activation(
                        out=h2T.rearrange("p (u c) -> p u c", c=C1 + 1)[
                            :, v0:v1, 0:C1],
                        in_=ps[:, 0:(v1 - v0) * C1].rearrange(
                            "p (u c) -> p u c", c=C1),
                        func=mybir.ActivationFunctionType.Relu,
                    )

            # ---------------- fc2: mult + segment-reduce on DVE -----------
            for v0, v1 in FC2G:
                w = (v1 - v0) * (C1 + 1)
                nc.vector.tensor_mul(
                    out=junk[:, 0:w],
                    in0=h2T[:, v0 * (C1 + 1):v1 * (C1 + 1)],
                    in1=w2rep_sb[:, v0 * (C1 + 1):v1 * (C1 + 1)],
                )
                nc.vector.reduce_sum(
                    out=h3[:, v0:v1],
                    in_=junk[:, 0:w].rearrange("p (u c) -> p u c", c=C1 + 1),
                    axis=mybir.AxisListType.X,
                )

            # BN3 relu (affine folded into w2rep/c3), then final linear
            nc.vector.tensor_scalar_max(out=h3r, in0=h3, scalar1=0.0)
            for cls in range(NCLS):
                nc.vector.tensor_mul(
                    out=junk[:, 0:ULOC],
                    in0=h3r,
                    in1=fwrep_sb[:, cls * ULOC:(cls + 1) * ULOC],
                )
                nc.vector.reduce_sum(
                    out=out_sb[:, cls:cls + 1], in_=junk[:, 0:ULOC],
                    axis=mybir.AxisListType.X,
                )
            nc.sync.dma_start(out=out_part[:, :], in_=out_sb)

    nc.finalize()
    return nc


def _host_prep(inputs):
    """Fold BN affines, pad units to 304, build per-core input maps."""
    x = np.asarray(inputs["x"], np.float32)
    conv_w = np.asarray(inputs["conv_w"], np.float32)
    conv_b = np.asarray(inputs["conv_b"], np.float32)
    g1, b1 = np.asarray(inputs["bn1_g"], np.float32), np.asarray(inputs["bn1_b"], np.float32)
    m1, v1 = np.asarray(inputs["bn1_m"], np.float32), np.asarray(inputs["bn1_v"], np.float32)
    fc1_w, fc1_b = np.asarray(inputs["fc1_w"], np.float32), np.asarray(inputs["fc1_b"], np.float32)
    g2, b2 = np.asarray(inputs["bn2_g"], np.float32), np.asarray(inputs["bn2_b"], np.float32)
    m2, v2 = np.asarray(inputs["bn2_m"], np.float32), np.asarray(inputs["bn2_v"], np.float32)
    fc2_w, fc2_b = np.asarray(inputs["fc2_w"], np.float32), np.asarray(inputs["fc2_b"], np.float32)
    g3, b3 = np.asarray(inputs["bn3_g"], np.float32), np.asarray(inputs["bn3_b"], np.float32)
    m3, v3 = np.asarray(inputs["bn3_m"], np.float32), np.asarray(inputs["bn3_v"], np.float32)
    final_w = np.asarray(inputs["final_w"], np.float32)
    final_b = np.asarray(inputs["final_b"], np.float32)

    a1 = g1 / np.sqrt(v1 + EPS)                      # [300] > 0
    c1 = a1 * (conv_b - m1) + b1                     # [300]
    a2 = g2 / np.sqrt(v2 + EPS)                      # [300,100]
    c2 = b2 - a2 * m2 + a2 * fc1_b                   # [300,100]
    a3 = g3 / np.sqrt(v3 + EPS)                      # [300]
    c3 = a3 * (fc2_b - m3) + b3                      # [300]

    bf = ml_dtypes.bfloat16

    wconv = np.zeros((CK, NPAD), np.float32)
    wconv[:, :N] = conv_w.transpose(1, 2, 0).reshape(CK, N)

    a1p = np.ones(NPAD, np.float32)
    c1p = np.zeros(NPAD, np.float32)
    a1p[:N], c1p[:N] = a1, c1
    a1c1 = np.zeros((128, 6), np.float32)
    a1c1[:, 0:3] = 1.0
    for ci, (u0, P, usz) in enumerate(UCHUNKS):
        a1c1[0:P, ci] = a1p[u0:u0 + P]
        a1c1[0:P, 3 + ci] = c1p[u0:u0 + P]

    w1mod = np.zeros((NPAD, C1, LP), np.float32)
    w1mod[:N] = fc1_w * a2[:, :, None]
    c2p = np.zeros((NPAD, C1), np.float32)
    c2p[:N] = c2
    w2mod = np.zeros((NPAD, C1), np.float32)
    w2mod[:N] = fc2_w * a3[:, None]
    c3p = np.zeros(NPAD, np.float32)
    c3p[:N] = c3
    fwp = np.zeros((NCLS, NPAD), np.float32)
    fwp[:, :N] = final_w

    ident = np.eye(128, dtype=np.float32)

    in_maps = []
    for j in range(NCORES):
        # owned global units in UCHUNKS order (small chunk first)
        gs = []
        for ci, (u0, P, usz) in enumerate(UCHUNKS):
            gs += [u0 + j * usz + v for v in range(usz)]
        # fc1: per owned unit [84, 100]: rows 0..82 = a2*w1 (p-major),
        # row 83 = c2 (pairs with the pooled ones-row)
        w1c = np.zeros((LP + 1, ULOC * C1), np.float32)
        # fc2: per owned unit 101 cols: a3*w2 then c3 (pairs with h2T ones-col)
        w2c = np.zeros((ULOC * (C1 + 1),), np.float32)
        fwc = np.zeros((NCLS, ULOC), np.float32)
        for v, g in enumerate(gs):
            w1c[0:LP, v * C1:(v + 1) * C1] = w1mod[g].T
            w1c[LP, v * C1:(v + 1) * C1] = c2p[g]
            w2c[v * (C1 + 1):v * (C1 + 1) + C1] = w2mod[g]
            w2c[v * (C1 + 1) + C1] = c3p[g]
            fwc[:, v] = fwp[:, g]
        # im2col [c*19+k, b, l] = x[b, c, k+l]
        xc = x[j * BLOC:(j + 1) * BLOC]
        sw = np.lib.stride_tricks.sliding_window_view(xc, LC, axis=2)  # [16,4,20,581]
        xim = np.ascontiguousarray(
            sw[:, :, :K, :].transpose(1, 2, 0, 3).reshape(CK, BLOC, LC))
        in_maps.append({
            "xim": xim.astype(bf),
            "wconv": wconv.astype(bf),
            "a1c1": a1c1,
            "ident": ident.astype(bf),
            "w1": w1c.astype(bf),
            "w2row": w2c.astype(bf)[None, :],
            "fwrow": fwc.reshape(1, -1),
        })
    return in_maps, final_b


def kernel(**inputs):
    from concourse.bass_utils import run_bass_kernel_spmd

    if "nc" not in _CACHE:
        _CACHE["nc"] = _build_bass()
    nc = _CACHE["nc"]

    in_maps, final_b = _host_prep(inputs)
    res = run_bass_kernel_spmd(nc, in_maps, core_ids=list(range(NCORES)))
    out = np.zeros((B, NCLS), np.float32)
    for r in res.results:
        out += r["out_part"]
    out += final_b[None, :]
    return out
